# revision 22
# baseline (speedup 1.0000x reference)
"""GCN-5 message-passing kernel for Trainium2, 8-core SPMD Bass/Tile.

Strategy (graph-parallel per the sharding hint):
  - Graphs are assigned to the 8 cores by a greedy balance on edge load;
    each core owns whole graphs, its nodes, and the edges terminating in
    them (plus their self-loops).  Pool/LN/head are local per core.
  - Per layer each core computes z = h @ W for its node slice (bf16,
    rows padded to 256B), the slices are AllGathered in two row-chunks
    (overlapping the producing tile loop), and each core gathers z[src]
    rows for its edges with gpsimd dma_gather: batched descriptor
    generation (<=1024 rows per call, single_packet) spread across the 4
    SWDGE queues so all four Q7 pairs generate descriptors in parallel.
  - Scatter-add per 128-node tile is a one-hot matmul on the
    TensorEngine with PSUM accumulation (edges pre-sorted by dst tile /
    src region and padded to 128-slot blocks on the host; padded slots
    carry nrm=0 so they contribute nothing).  The symmetric norm
    deg^-1/2(src)*deg^-1/2(dst) is a per-slot bf16 plane folded into one
    vector multiply on the gathered rows.
  - All matmul operands are bf16 (PSUM accumulation fp32); gather and
    collective payloads are bf16.  Final pool/LN/head tail is fp32.
"""
import sys
import types
import contextlib

import numpy as np
import ml_dtypes

sys.path.insert(0, "/opt/trn_rl_repo")

import concourse.bass as bass
import concourse.tile as tile
from concourse import mybir
from concourse.vector_clock import ScopedClock

F32 = mybir.dt.float32
BF16 = mybir.dt.bfloat16
I16 = mybir.dt.int16
M = 8   # NeuronCores
H = 64
N_GRAPHS = 1000
TPC = 8       # dst tiles per gather chunk
MAXIDX = 1024  # max rows per dma_gather call (16KB/engine packet limit)

# ---------------------------------------------------------------------------
# Environment fixes for this container
# ---------------------------------------------------------------------------

def _install_env_fixes():
    import concourse.tile as tile_mod

    def _patched_drain_and_barrier(self, tick_clock, wait_clock):
        # this walrus build allows a single sync-wait per TPB_CTRL Drain;
        # split the Tile tail-drain's waits across multiple drains.
        nc = self.nc
        drain_inst = nc.sync.drain()
        wait_clock.add_sem_waits(drain_inst.ins,
                                 ScopedClock({None: tick_clock.global_clock}))
        si = drain_inst.ins.sync_info
        waits = list(si.on_wait or [])
        if len(waits) > 1:
            si.on_wait[:] = waits[:1]
            for w in waits[1:]:
                d2 = nc.sync.drain()
                if d2.ins.sync_info is None:
                    d2.ins.sync_info = mybir.SyncInfo(on_wait=[w], on_update=[])
                else:
                    d2.ins.sync_info.on_wait.append(w)
        nc.all_engine_barrier()
        assert self.sems is not None
        popped = nc._tile_sem_poison_stack.pop()
        assert popped is self._sem_poison
        nc.clear_and_free_semaphores(list(self.sems.allocated().values()))
        nc.all_engine_barrier()

    tile_mod.TileContext._drain_and_barrier = _patched_drain_and_barrier

    _orig_lower_ordered = tile_mod.TileContext._lower_ordered_insts

    def _split_multiwait_lower(self, ordered):
        nc = self.nc
        for bbname, insts in ordered.items():
            newlist = []
            changed = False
            for inst in insts:
                si = getattr(inst, "sync_info", None)
                eng = getattr(inst, "engine", None)
                if (si is not None and si.on_wait and len(si.on_wait) > 1
                        and eng is not None and eng != mybir.EngineType.Unassigned
                        and inst.is_executable()):
                    waits = list(si.on_wait)
                    si.on_wait[:] = waits[-1:]
                    for w in waits[:-1]:
                        nop = mybir.InstNoOp(
                            name=nc.get_next_instruction_name(), engine=eng)
                        nop.sync_info = mybir.SyncInfo(on_wait=[w], on_update=[])
                        try:
                            nc.register_instruction(nop, overwrite=True)
                        except Exception:
                            pass
                        newlist.append(nop)
                    changed = True
                newlist.append(inst)
            if changed:
                insts[:] = newlist
        return _orig_lower_ordered(self, ordered)

    if getattr(tile_mod.TileContext._lower_ordered_insts, "__name__", "") != \
            "_split_multiwait_lower":
        tile_mod.TileContext._lower_ordered_insts = _split_multiwait_lower

    # NTFF profile hook (lets trace=True work under axon); best-effort.
    if "antenv.axon_hooks" not in sys.modules:
        try:
            from trn_agent_boot.trn_boot import _ntff_profile_via_ctypes
            hook = _ntff_profile_via_ctypes("/opt/axon/libaxon_pjrt.so")
            mod = types.ModuleType("antenv.axon_hooks")
            mod.get_axon_ntff_profile_hook = lambda: hook
            mod.set_axon_ntff_profile_hook = lambda h: None
            sys.modules["antenv.axon_hooks"] = mod
            import antenv
            antenv.axon_hooks = mod
        except Exception:
            pass


_install_env_fixes()


def _bf16(a):
    return np.ascontiguousarray(np.asarray(a, np.float32)).astype(ml_dtypes.bfloat16)


# ---------------------------------------------------------------------------
# Host preprocessing
# ---------------------------------------------------------------------------

def preprocess(x, edge_index, batch):
    N = x.shape[0]
    src0 = edge_index[0].astype(np.int64)
    dst0 = edge_index[1].astype(np.int64)
    loops = np.arange(N, dtype=np.int64)
    src = np.concatenate([src0, loops])
    dst = np.concatenate([dst0, loops])
    deg = np.bincount(dst, minlength=N).astype(np.float32)
    dis = 1.0 / np.sqrt(deg)          # deg >= 1 (self loops)
    batch = np.asarray(batch).astype(np.int64)

    # ---- graph -> core assignment (greedy balance on edge load) ----
    cnt_g = np.bincount(batch, minlength=N_GRAPHS).astype(np.int64)
    eload = np.bincount(batch[dst], minlength=N_GRAPHS).astype(np.int64)
    order = np.argsort(-eload, kind="stable")
    core_e = np.zeros(M, np.int64)
    core_n = np.zeros(M, np.int64)
    core_g = np.zeros(M, np.int64)
    graph_core = np.zeros(N_GRAPHS, np.int64)
    node_cap = int(np.ceil(N / M / 128.0 + 1)) * 128
    for g in order:
        cands = np.nonzero((core_n + cnt_g[g] <= node_cap)
                           & (core_g < 128))[0]
        if len(cands) == 0:
            cands = np.nonzero(core_g < 128)[0]
        c = cands[np.argmin(core_e[cands])]
        graph_core[g] = c
        core_e[c] += eload[g]
        core_n[c] += cnt_g[g]
        core_g[c] += 1
    T = int(np.ceil(core_n.max() / 128.0))
    T2 = (T + 1) // 2    # int16 row-index limit: 4*T2*128 <= 32767
    S = T * 128

    core_graphs = [np.nonzero(graph_core == c)[0] for c in range(M)]
    node_core = graph_core[batch]

    # ---- node -> (tile, pos) within core: balance per-tile in-edge load ----
    indeg = np.bincount(dst, minlength=N).astype(np.int64)  # includes self
    tile_of = np.zeros(N, np.int64)
    pos_of = np.zeros(N, np.int64)
    glocal = np.zeros(N, np.int64)
    import heapq
    for c in range(M):
        gl = {g: i for i, g in enumerate(core_graphs[c])}
        nodes = np.nonzero(node_core == c)[0]
        glocal[nodes] = [gl[g] for g in batch[nodes]]
        nd = nodes[np.argsort(-indeg[nodes], kind="stable")]
        heap = [(0, 0, t) for t in range(T)]
        heapq.heapify(heap)
        counts = np.zeros(T, np.int64)
        for n in nd:
            load, cnt, t = heapq.heappop(heap)
            tile_of[n] = t
            pos_of[n] = counts[t]
            counts[t] += 1
            if counts[t] < 128:
                heapq.heappush(heap, (load + indeg[n], counts[t], t))

    # ---- zfull row / region layout (2 collective chunks x 2 core halves) ---
    n_chunk0 = M * T2 * 128
    tpc1 = T - T2

    def zrow(nodes):
        c = node_core[nodes]
        t = tile_of[nodes]
        p = pos_of[nodes]
        in0 = t < T2
        r = np.where(in0,
                     c * (T2 * 128) + t * 128 + p,
                     n_chunk0 + c * (tpc1 * 128) + (t - T2) * 128 + p)
        return r

    reg_sizes = np.array([4 * T2 * 128, 4 * T2 * 128,
                          4 * tpc1 * 128, 4 * tpc1 * 128], np.int64)
    reg_base = np.concatenate([[0], np.cumsum(reg_sizes)])[:4]
    assert reg_sizes.max() <= 32767, reg_sizes

    def region(nodes):
        c = node_core[nodes]
        t = tile_of[nodes]
        return np.where(t < T2, 0, 2) + (c >= 4).astype(np.int64)

    src_row = zrow(src)
    src_reg = region(src)
    nrm_e = (dis[src] * dis[dst]).astype(np.float32)
    e_core = node_core[dst]
    e_tile = tile_of[dst]
    e_pos = pos_of[dst]

    # ---- group edges by (core, tile, region); common block counts ----
    NC = int(np.ceil(T / TPC))
    counts = np.zeros((M, T, 4), np.int64)
    np.add.at(counts, (e_core, e_tile, src_reg), 1)
    B_tr = np.ceil(counts.max(axis=0) / 128.0).astype(np.int64)  # [T, 4]
    B_tr = np.maximum(B_tr, 1)

    # global block order: chunk-major, then region, then tile, then block
    blk_cols = {}
    col = 0
    nblk_chunk = np.zeros(NC, np.int64)
    for ch in range(NC):
        t0, t1 = ch * TPC, min((ch + 1) * TPC, T)
        base = col
        for r in range(4):
            for t in range(t0, t1):
                blk_cols[(t, r)] = col
                col += B_tr[t, r]
        nblk_chunk[ch] = col - base
    TOTB = col
    NBLKC = int(nblk_chunk.max())
    TOTS = TOTB * 128

    # call plan: per (chunk, region) a list of (colbase, nblk<=8) subcalls
    calls = []
    for ch in range(NC):
        t0, t1 = ch * TPC, min((ch + 1) * TPC, T)
        cb = int(nblk_chunk[:ch].sum())
        for r in range(4):
            nb = int(B_tr[t0:t1, r].sum())
            first = blk_cols[(t0, r)]
            done = 0
            while done < nb:
                k = min(nb - done, MAXIDX // 128)
                calls.append(dict(chunk=ch, region=r,
                                  goff=first - cb + done - 0,
                                  gcol=first + done, nblk=k))
                done += k

    # per-tile matmul block lists (chunk-local g columns)
    tile_blocks = []  # [t] -> list of chunk-local block offsets
    for t in range(T):
        ch = t // TPC
        cb = int(nblk_chunk[:ch].sum())
        bl = []
        for r in range(4):
            c0 = blk_cols[(t, r)]
            bl.extend(range(c0 - cb, c0 - cb + int(B_tr[t, r])))
        tile_blocks.append(bl)

    # ---- per-core slot planes ----
    in_maps = []
    for c in range(M):
        sel = e_core == c
        et, er, epos_c = e_tile[sel], src_reg[sel], e_pos[sel]
        esrc_local = (src_row[sel] - reg_base[er]).astype(np.int64)
        enrm = nrm_e[sel]
        okey = et * 4 + er
        order_e = np.argsort(okey, kind="stable")
        et, er, epos_c = et[order_e], er[order_e], epos_c[order_e]
        esrc_local, enrm = esrc_local[order_e], enrm[order_e]
        cnt_tr = np.zeros((T, 4), np.int64)
        np.add.at(cnt_tr, (et, er), 1)
        cum = np.zeros((T, 4), np.int64)
        flat = cnt_tr.reshape(-1)
        cumflat = np.concatenate([[0], np.cumsum(flat)])[:-1]
        cum = cumflat.reshape(T, 4)
        # slot index within (t, r) group:
        within = np.arange(len(et)) - cum[et, er]
        gblk = np.array([blk_cols[(int(t), int(r))] for t, r in zip(et, er)])
        slot = gblk * 128 + within
        idxplane = np.zeros(TOTS, np.int64)
        nrmplane = np.zeros(TOTS, np.float32)
        ldstplane = np.full(TOTS, -1.0, np.float32)
        idxplane[slot] = esrc_local
        nrmplane[slot] = enrm
        ldstplane[slot] = epos_c

        # idx wrapped layout [128, TOTS//16] int16, replicated per 16-part grp
        wrapped = idxplane.reshape(TOTS // 16, 16).T.astype(np.int16)
        idx_sb = np.zeros((128, TOTS // 16), np.int16)
        for k in range(8):
            idx_sb[16 * k:16 * (k + 1), :] = wrapped

        def to_cols(a):  # slot plane -> [128, TOTB] (partition = slot % 128)
            return np.ascontiguousarray(a.reshape(TOTB, 128).T)

        nodes = np.nonzero(node_core == c)[0]
        xT = np.zeros((128, S), np.float32)
        scol = tile_of[nodes] * 128 + pos_of[nodes]
        xT[:, scol] = x[nodes].T
        gcolT = np.full((128, T), -1.0, np.float32)
        gcolT[pos_of[nodes], tile_of[nodes]] = glocal[nodes]
        discol = np.zeros((128, T), np.float32)
        discol[pos_of[nodes], tile_of[nodes]] = dis[nodes]

        gpc = len(core_graphs[c])
        invcnt = np.zeros((128, 1), np.float32)
        invcnt[:gpc, 0] = 1.0 / np.maximum(cnt_g[core_graphs[c]], 1.0)

        in_maps.append(dict(
            xT=_bf16(xT),
            idxp=idx_sb,
            nrmp=_bf16(to_cols(nrmplane)),
            ldstp=_bf16(to_cols(ldstplane)),
            gcolT=_bf16(gcolT),
            invcnt=invcnt.astype(np.float32),
        ))

    iota = np.broadcast_to(np.arange(128, dtype=np.float32), (128, 128)).copy()
    ident = np.eye(128, dtype=np.float32)
    shared = dict(iota=_bf16(iota), ident=ident)

    meta = dict(T=T, T2=T2, S=S, NC=NC, TOTB=TOTB, TOTS=TOTS, NBLKC=NBLKC,
                nblk_chunk=nblk_chunk.tolist(), calls=calls,
                tile_blocks=tile_blocks, reg_base=reg_base.tolist(),
                reg_sizes=reg_sizes.tolist(), n_chunk0=n_chunk0,
                core_graphs=core_graphs, shared=shared)
    return in_maps, meta


def make_weight_inputs(W1, b1, Wh, bh, Wout, bout):
    b14 = np.stack([np.asarray(b1, np.float32)] +
                   [np.asarray(bh[i], np.float32) for i in range(3)], axis=1)
    return dict(
        W1=_bf16(W1),
        Wh2=_bf16(np.concatenate([np.asarray(Wh[i]) for i in range(4)],
                                 axis=1)),
        b14=np.ascontiguousarray(b14, np.float32),
        b5rep=np.broadcast_to(np.asarray(bh[3], np.float32), (128, H)).copy(),
        woutrep=np.broadcast_to(np.asarray(Wout, np.float32)[:, 0],
                                (128, H)).copy(),
    ), float(np.asarray(bout, np.float32)[0])


# ---------------------------------------------------------------------------
# Bass program
# ---------------------------------------------------------------------------

def build_nc(meta, bout):
    from concourse.library_config import mlp

    T, T2, S, NC = meta["T"], meta["T2"], meta["S"], meta["NC"]
    TOTB, TOTS, NBLKC = meta["TOTB"], meta["TOTS"], meta["NBLKC"]
    nblk_chunk = meta["nblk_chunk"]
    calls = meta["calls"]
    tile_blocks = meta["tile_blocks"]
    reg_base = meta["reg_base"]
    reg_sizes = meta["reg_sizes"]
    n_chunk0 = meta["n_chunk0"]
    ZR = M * S

    nc = bass.Bass("TRN2", target_bir_lowering=False, num_swdge_queues=4)

    xT_d = nc.declare_dram_parameter("xT", [128, S], BF16, isOutput=False)
    idxp_d = nc.declare_dram_parameter("idxp", [128, TOTS // 16], I16, isOutput=False)
    nrmp_d = nc.declare_dram_parameter("nrmp", [128, TOTB], BF16, isOutput=False)
    ldstp_d = nc.declare_dram_parameter("ldstp", [128, TOTB], BF16, isOutput=False)
    gcolT_d = nc.declare_dram_parameter("gcolT", [128, T], BF16, isOutput=False)
    invcnt_d = nc.declare_dram_parameter("invcnt", [128, 1], F32, isOutput=False)
    iota_d = nc.declare_dram_parameter("iota", [128, 128], BF16, isOutput=False)
    ident_d = nc.declare_dram_parameter("ident", [128, 128], F32, isOutput=False)
    W1_d = nc.declare_dram_parameter("W1", [128, H], BF16, isOutput=False)
    Wh2_d = nc.declare_dram_parameter("Wh2", [H, 4 * H], BF16, isOutput=False)
    b14_d = nc.declare_dram_parameter("b14", [H, 4], F32, isOutput=False)
    b5rep_d = nc.declare_dram_parameter("b5rep", [128, H], F32, isOutput=False)
    woutrep_d = nc.declare_dram_parameter("woutrep", [128, H], F32, isOutput=False)
    out_d = nc.declare_dram_parameter("out", [128, 1], F32, isOutput=True)

    with tile.TileContext(nc) as tc:
        with contextlib.ExitStack() as ctx:
            dram = ctx.enter_context(tc.tile_pool(name="dram", bufs=1, space="DRAM"))
            zpool = ctx.enter_context(tc.tile_pool(name="zpool", bufs=1, space="DRAM"))
            const = ctx.enter_context(tc.tile_pool(name="const", bufs=1))
            xp = ctx.enter_context(tc.tile_pool(name="xp", bufs=3))
            gp = ctx.enter_context(tc.tile_pool(name="gp", bufs=4))
            mp = ctx.enter_context(tc.tile_pool(name="mp", bufs=2))
            hp = ctx.enter_context(tc.tile_pool(name="hp", bufs=3))
            zp = ctx.enter_context(tc.tile_pool(name="zp", bufs=3))
            ep = ctx.enter_context(tc.tile_pool(name="ep", bufs=2))
            ps_agg = ctx.enter_context(tc.tile_pool(name="ps_agg", bufs=3, space="PSUM"))
            ps_z = ctx.enter_context(tc.tile_pool(name="ps_z", bufs=2, space="PSUM"))
            ps_pool = ctx.enter_context(tc.tile_pool(name="ps_pool", bufs=1, space="PSUM"))

            nc.gpsimd.load_library(mlp)

            zfullsA = [zpool.tile([n_chunk0, 128], BF16, addr_space="Shared",
                                  name=f"zfullA{k}", tag=f"zfullA{k}")
                       for k in range(5)]
            zfullsB = [zpool.tile([ZR - n_chunk0, 128], BF16,
                                  addr_space="Shared",
                                  name=f"zfullB{k}", tag=f"zfullB{k}")
                       for k in range(5)]
            bounces = [dram.tile([S, 128], BF16, name=f"bounce{k}",
                                 tag=f"bounce{k}") for k in range(5)]

            idx_sb = const.tile([128, TOTS // 16], I16)
            nc.sync.dma_start(idx_sb[:], idxp_d[:])
            nrm_sb = const.tile([128, TOTB], BF16)
            nc.sync.dma_start(nrm_sb[:], nrmp_d[:])
            ldst_sb = const.tile([128, TOTB], BF16)
            nc.sync.dma_start(ldst_sb[:], ldstp_d[:])
            gcol_sb = const.tile([128, T], BF16)
            nc.sync.dma_start(gcol_sb[:], gcolT_d[:])
            invcnt_sb = const.tile([128, 1], F32)
            nc.sync.dma_start(invcnt_sb[:], invcnt_d[:])
            iota_sb = const.tile([128, 128], BF16)
            nc.sync.dma_start(iota_sb[:], iota_d[:])
            ident_sb = const.tile([128, 128], F32)
            nc.sync.dma_start(ident_sb[:], ident_d[:])
            W1_sb = const.tile([128, H], BF16)
            nc.sync.dma_start(W1_sb[:], W1_d[:])
            Wh2_sb = const.tile([H, 4 * H], BF16)
            nc.sync.dma_start(Wh2_sb[:], Wh2_d[:])
            b14_sb = const.tile([H, 4], F32)
            nc.sync.dma_start(b14_sb[:], b14_d[:])
            b5rep_sb = const.tile([128, H], F32)
            nc.sync.dma_start(b5rep_sb[:], b5rep_d[:])
            woutrep_sb = const.tile([128, H], F32)
            nc.sync.dma_start(woutrep_sb[:], woutrep_d[:])

            nblk_vals = sorted({cl["nblk"] for cl in calls})
            nidx_regs = {nb: nc.gpsimd.to_reg(nb * 128) for nb in nblk_vals}

            def all_gather_A(layer):
                bo = bounces[layer]
                nc.gpsimd.collective_compute(
                    "AllGather", mybir.AluOpType.bypass,
                    replica_groups=[list(range(M))],
                    ins=[bo[0:T2 * 128, :]], outs=[zfullsA[layer][:]])

            def all_gather_B(layer):
                bo = bounces[layer]
                if T > T2:
                    nc.gpsimd.collective_compute(
                        "AllGather", mybir.AluOpType.bypass,
                        replica_groups=[list(range(M))],
                        ins=[bo[T2 * 128:S, :]], outs=[zfullsB[layer][:]])

            # ---- layer 0: z1 = x @ W1 per tile ----
            for t in range(T):
                xt = xp.tile([128, 128], BF16, tag="xt")
                nc.sync.dma_start(xt[:], xT_d[:, t * 128:(t + 1) * 128])
                pz = ps_z.tile([128, H], F32, space="PSUM", tag="pz")
                nc.tensor.matmul(out=pz[:], lhsT=xt[:], rhs=W1_sb[:],
                                 start=True, stop=True)
                zt = zp.tile([128, H], BF16, tag="zt")
                nc.scalar.copy(zt[:], pz[:])
                nc.sync.dma_start(bounces[0][t * 128:(t + 1) * 128, 0:H], zt[:])
                if t == T2 - 1:
                    all_gather_A(0)
            all_gather_B(0)

            # ---- layers 1..5 ----
            chunk_call = [[cl for cl in calls if cl["chunk"] == ch]
                          for ch in range(NC)]
            def emit_call(layer, ch, g, cl):
                cb = int(np.sum(nblk_chunk[:ch]))
                r = cl["region"]
                go = cl["gcol"] - cb
                if r < 2:
                    zf, rb = zfullsA[layer - 1], reg_base[r]
                else:
                    zf, rb = zfullsB[layer - 1], reg_base[r] - n_chunk0
                rs = reg_sizes[r]
                nc.gpsimd.dma_gather(
                    g[:, go:go + cl["nblk"], :],
                    zf[rb:rb + rs, :],
                    idx_sb[:, cl["gcol"] * 8:(cl["gcol"] + cl["nblk"]) * 8],
                    cl["nblk"] * 128, nidx_regs[cl["nblk"]], 128,
                    single_packet=True, queue_num=r)

            def issue_calls(layer, specs):
                # specs: list of (ch, g, regions); interleave round-robin
                # across queues so the 8-deep engine window spans all pairs.
                lanes = {0: [], 1: [], 2: [], 3: []}
                for ch, g, regions in specs:
                    for cl in chunk_call[ch]:
                        if cl["region"] in regions:
                            lanes[cl["region"]].append((ch, g, cl))
                alive = True
                while alive:
                    alive = False
                    for r in range(4):
                        if lanes[r]:
                            ch, g, cl = lanes[r].pop(0)
                            emit_call(layer, ch, g, cl)
                            alive = True

            AHEAD = 2  # chunks of regions-0/1 gathers issued ahead
            for layer in range(1, 6):
                if layer == 5:
                    ppool = ps_pool.tile([H, 128], F32, space="PSUM", tag="pp")
                gtiles = {}
                for step in range(NC + AHEAD):
                    specs = []
                    if step < NC:
                        g = gp.tile([128, NBLKC, 128], BF16, tag="g")
                        gtiles[step] = g
                        specs.append((step, g, (0, 1)))
                    if step >= AHEAD:
                        specs.append((step - AHEAD, gtiles[step - AHEAD], (2, 3)))
                    issue_calls(layer, specs)
                    if step < AHEAD:
                        continue
                    ch = step - AHEAD
                    g = gtiles.pop(ch)
                    cb = int(np.sum(nblk_chunk[:ch]))
                    nblk = int(nblk_chunk[ch])
                    # norm scale on live halves
                    g3 = g[:, 0:nblk, 0:H]
                    nrm3 = nrm_sb[:, cb:cb + nblk, None].to_broadcast(
                        [128, nblk, H])
                    nc.vector.tensor_tensor(out=g3, in0=g3, in1=nrm3,
                                            op=mybir.AluOpType.mult)
                    # one-hot build for the whole chunk
                    m01 = mp.tile([128, NBLKC, 128], BF16, tag="m01")
                    m3 = m01[:, 0:nblk, :]
                    iota3 = iota_sb[:, None, :].to_broadcast([128, nblk, 128])
                    ldst3 = ldst_sb[:, cb:cb + nblk, None].to_broadcast(
                        [128, nblk, 128])
                    nc.vector.tensor_tensor(out=m3, in0=iota3, in1=ldst3,
                                            op=mybir.AluOpType.is_equal)
                    t0, t1 = ch * TPC, min((ch + 1) * TPC, T)
                    for t in range(t0, t1):
                        bl = tile_blocks[t]
                        if layer < 5:
                            pT = ps_agg.tile([H, 128], F32, space="PSUM", tag="pT")
                            for i, b in enumerate(bl):
                                nc.tensor.matmul(
                                    out=pT[:], lhsT=g[:, b, 0:H],
                                    rhs=m01[:, b, :],
                                    start=(i == 0), stop=(i == len(bl) - 1))
                            hT = hp.tile([H, 128], BF16, tag="hT")
                            nc.scalar.activation(
                                hT[:], pT[:], mybir.ActivationFunctionType.Relu,
                                bias=b14_sb[:, layer - 1:layer])
                            pz = ps_z.tile([128, H], F32, space="PSUM", tag="pz")
                            nc.tensor.matmul(
                                out=pz[:], lhsT=hT[:],
                                rhs=Wh2_sb[:, (layer - 1) * H:layer * H],
                                start=True, stop=True)
                            zt = zp.tile([128, H], BF16, tag="zt2")
                            nc.scalar.copy(zt[:], pz[:])
                            nc.sync.dma_start(
                                bounces[layer][t * 128:(t + 1) * 128, 0:H], zt[:])
                        else:
                            p5 = ps_agg.tile([128, H], F32, space="PSUM", tag="pT")
                            for i, b in enumerate(bl):
                                nc.tensor.matmul(
                                    out=p5[:], lhsT=m01[:, b, :],
                                    rhs=g[:, b, 0:H],
                                    start=(i == 0), stop=(i == len(bl) - 1))
                            h5 = hp.tile([128, H], BF16, tag="h5")
                            nc.vector.tensor_tensor(
                                out=h5[:], in0=p5[:], in1=b5rep_sb[:],
                                op=mybir.AluOpType.add)
                            nc.scalar.activation(
                                h5[:], h5[:], mybir.ActivationFunctionType.Relu)
                            pt = mp.tile([128, 128], BF16, tag="pt")
                            nc.vector.tensor_tensor(
                                out=pt[:], in0=iota_sb[:],
                                in1=gcol_sb[:, t:t + 1].to_broadcast([128, 128]),
                                op=mybir.AluOpType.is_equal)
                            nc.tensor.matmul(out=ppool[:], lhsT=h5[:], rhs=pt[:],
                                             start=(t == 0), stop=(t == T - 1))
                    if layer < 5 and ch == (T2 + TPC - 1) // TPC - 1:
                        all_gather_A(layer)
                if layer < 5:
                    all_gather_B(layer)

            # ---- pool tail: mean, layernorm, head (fp32) ----
            pool_sb = ep.tile([H, 128], F32, tag="poolsb")
            nc.scalar.copy(pool_sb[:], ppool[:])
            ptr = ps_z.tile([128, H], F32, space="PSUM", tag="ptr")
            nc.tensor.transpose(out=ptr[:], in_=pool_sb[:],
                                identity=ident_sb[0:H, 0:H])
            pooled = ep.tile([128, H], F32, tag="pooled")
            nc.vector.tensor_scalar(out=pooled[:], in0=ptr[:],
                                    scalar1=invcnt_sb[:, 0:1], scalar2=None,
                                    op0=mybir.AluOpType.mult)
            mu = ep.tile([128, 1], F32, tag="mu")
            nc.vector.tensor_reduce(out=mu[:], in_=pooled[:],
                                    axis=mybir.AxisListType.X,
                                    op=mybir.AluOpType.add)
            nc.vector.tensor_scalar(out=mu[:], in0=mu[:], scalar1=1.0 / H,
                                    scalar2=None, op0=mybir.AluOpType.mult)
            xc = ep.tile([128, H], F32, tag="xc")
            nc.vector.tensor_scalar(out=xc[:], in0=pooled[:], scalar1=mu[:, 0:1],
                                    scalar2=None, op0=mybir.AluOpType.subtract)
            sq = ep.tile([128, H], F32, tag="sq")
            nc.scalar.activation(sq[:], xc[:], mybir.ActivationFunctionType.Square)
            var = ep.tile([128, 1], F32, tag="var")
            nc.vector.tensor_reduce(out=var[:], in_=sq[:],
                                    axis=mybir.AxisListType.X,
                                    op=mybir.AluOpType.add)
            nc.vector.tensor_scalar(out=var[:], in0=var[:], scalar1=1.0 / H,
                                    scalar2=None, op0=mybir.AluOpType.mult)
            eps_col = ep.tile([128, 1], F32, tag="eps")
            nc.vector.memset(eps_col[:], 1e-5)
            std = ep.tile([128, 1], F32, tag="std")
            nc.scalar.activation(std[:], var[:], mybir.ActivationFunctionType.Sqrt,
                                 bias=eps_col[:, 0:1])
            rstd = ep.tile([128, 1], F32, tag="rstd")
            nc.vector.reciprocal(rstd[:], std[:])
            ln = ep.tile([128, H], F32, tag="ln")
            nc.vector.tensor_scalar(out=ln[:], in0=xc[:], scalar1=rstd[:, 0:1],
                                    scalar2=None, op0=mybir.AluOpType.mult)
            y = ep.tile([128, H], F32, tag="y")
            nc.vector.tensor_tensor(out=y[:], in0=ln[:], in1=woutrep_sb[:],
                                    op=mybir.AluOpType.mult)
            yr = ep.tile([128, 1], F32, tag="yr")
            nc.vector.tensor_reduce(out=yr[:], in_=y[:],
                                    axis=mybir.AxisListType.X,
                                    op=mybir.AluOpType.add)
            nc.vector.tensor_scalar(out=yr[:], in0=yr[:], scalar1=bout,
                                    scalar2=None, op0=mybir.AluOpType.add)
            nc.sync.dma_start(out_d[:], yr[:])
    return nc


# ---------------------------------------------------------------------------
# Entry point
# ---------------------------------------------------------------------------

def kernel(x, edge_index, batch, W1, b1, Wh, bh, Wout, bout):
    from concourse.bass_utils import run_bass_kernel_spmd
    from concourse.library_overlay import lower_extended_insts

    x = np.asarray(x, np.float32)
    edge_index = np.asarray(edge_index)
    batch = np.asarray(batch)

    in_maps, meta = preprocess(x, edge_index, batch)
    wmaps, bout_v = make_weight_inputs(W1, b1, Wh, bh, Wout, bout)
    nc = build_nc(meta, bout_v)
    lower_extended_insts(nc)
    for im in in_maps:
        im.update(wmaps)
        im.update(meta["shared"])

    import time
    last_err = None
    for attempt in range(3):
        try:
            res = run_bass_kernel_spmd(nc, in_maps, core_ids=list(range(M)))
            break
        except Exception as e:  # transient terminal hiccups / device recovery
            last_err = e
            time.sleep(30 * (attempt + 1))
    else:
        raise last_err

    out = np.zeros((N_GRAPHS, 1), np.float32)
    for c in range(M):
        gl = meta["core_graphs"][c]
        out[gl, 0] = res.results[c]["out"][:len(gl), 0]
    return np.ascontiguousarray(out, np.float32)


# revision 23
# speedup vs baseline: 1.0553x; 1.0553x over previous
"""GCN-5 message-passing kernel for Trainium2, 8-core SPMD Bass/Tile.

Strategy (graph-parallel per the sharding hint):
  - Graphs are assigned to the 8 cores by a greedy balance on edge load;
    each core owns whole graphs, its nodes, and the edges terminating in
    them (plus their self-loops).  Pool/LN/head are local per core.
  - Per layer each core computes z = h @ W for its node slice (bf16,
    rows padded to 256B), the slices are AllGathered in two row-chunks
    (overlapping the producing tile loop), and each core gathers z[src]
    rows for its edges with gpsimd dma_gather: batched descriptor
    generation (<=1024 rows per call, single_packet) spread across the 4
    SWDGE queues so all four Q7 pairs generate descriptors in parallel.
  - Scatter-add per 128-node tile is a one-hot matmul on the
    TensorEngine with PSUM accumulation (edges pre-sorted by dst tile /
    src region and padded to 128-slot blocks on the host; padded slots
    carry nrm=0 so they contribute nothing).  The symmetric norm
    deg^-1/2(src)*deg^-1/2(dst) is a per-slot bf16 plane folded into one
    vector multiply on the gathered rows.
  - All matmul operands are bf16 (PSUM accumulation fp32); gather and
    collective payloads are bf16.  Final pool/LN/head tail is fp32.
"""
import sys
import types
import contextlib

import numpy as np
import ml_dtypes

sys.path.insert(0, "/opt/trn_rl_repo")

import concourse.bass as bass
import concourse.tile as tile
from concourse import mybir
from concourse.vector_clock import ScopedClock

F32 = mybir.dt.float32
BF16 = mybir.dt.bfloat16
I16 = mybir.dt.int16
M = 8   # NeuronCores
H = 64
N_GRAPHS = 1000
TPC = 8       # dst tiles per gather chunk
MAXIDX = 1024  # max rows per dma_gather call (16KB/engine packet limit)

# ---------------------------------------------------------------------------
# Environment fixes for this container
# ---------------------------------------------------------------------------

def _install_env_fixes():
    import concourse.tile as tile_mod

    def _patched_drain_and_barrier(self, tick_clock, wait_clock):
        # this walrus build allows a single sync-wait per TPB_CTRL Drain;
        # split the Tile tail-drain's waits across multiple drains.
        nc = self.nc
        drain_inst = nc.sync.drain()
        wait_clock.add_sem_waits(drain_inst.ins,
                                 ScopedClock({None: tick_clock.global_clock}))
        si = drain_inst.ins.sync_info
        waits = list(si.on_wait or [])
        if len(waits) > 1:
            si.on_wait[:] = waits[:1]
            for w in waits[1:]:
                d2 = nc.sync.drain()
                if d2.ins.sync_info is None:
                    d2.ins.sync_info = mybir.SyncInfo(on_wait=[w], on_update=[])
                else:
                    d2.ins.sync_info.on_wait.append(w)
        nc.all_engine_barrier()
        assert self.sems is not None
        popped = nc._tile_sem_poison_stack.pop()
        assert popped is self._sem_poison
        nc.clear_and_free_semaphores(list(self.sems.allocated().values()))
        nc.all_engine_barrier()

    tile_mod.TileContext._drain_and_barrier = _patched_drain_and_barrier

    _orig_lower_ordered = tile_mod.TileContext._lower_ordered_insts

    def _split_multiwait_lower(self, ordered):
        nc = self.nc
        for bbname, insts in ordered.items():
            newlist = []
            changed = False
            for inst in insts:
                si = getattr(inst, "sync_info", None)
                eng = getattr(inst, "engine", None)
                if (si is not None and si.on_wait and len(si.on_wait) > 1
                        and eng is not None and eng != mybir.EngineType.Unassigned
                        and inst.is_executable()):
                    waits = list(si.on_wait)
                    si.on_wait[:] = waits[-1:]
                    for w in waits[:-1]:
                        nop = mybir.InstNoOp(
                            name=nc.get_next_instruction_name(), engine=eng)
                        nop.sync_info = mybir.SyncInfo(on_wait=[w], on_update=[])
                        try:
                            nc.register_instruction(nop, overwrite=True)
                        except Exception:
                            pass
                        newlist.append(nop)
                    changed = True
                newlist.append(inst)
            if changed:
                insts[:] = newlist
        return _orig_lower_ordered(self, ordered)

    if getattr(tile_mod.TileContext._lower_ordered_insts, "__name__", "") != \
            "_split_multiwait_lower":
        tile_mod.TileContext._lower_ordered_insts = _split_multiwait_lower

    # NTFF profile hook (lets trace=True work under axon); best-effort.
    if "antenv.axon_hooks" not in sys.modules:
        try:
            from trn_agent_boot.trn_boot import _ntff_profile_via_ctypes
            hook = _ntff_profile_via_ctypes("/opt/axon/libaxon_pjrt.so")
            mod = types.ModuleType("antenv.axon_hooks")
            mod.get_axon_ntff_profile_hook = lambda: hook
            mod.set_axon_ntff_profile_hook = lambda h: None
            sys.modules["antenv.axon_hooks"] = mod
            import antenv
            antenv.axon_hooks = mod
        except Exception:
            pass


_install_env_fixes()


def _bf16(a):
    return np.ascontiguousarray(np.asarray(a, np.float32)).astype(ml_dtypes.bfloat16)


# ---------------------------------------------------------------------------
# Host preprocessing
# ---------------------------------------------------------------------------

def preprocess(x, edge_index, batch):
    N = x.shape[0]
    src0 = edge_index[0].astype(np.int64)
    dst0 = edge_index[1].astype(np.int64)
    loops = np.arange(N, dtype=np.int64)
    src = np.concatenate([src0, loops])
    dst = np.concatenate([dst0, loops])
    deg = np.bincount(dst, minlength=N).astype(np.float32)
    dis = 1.0 / np.sqrt(deg)          # deg >= 1 (self loops)
    batch = np.asarray(batch).astype(np.int64)

    # ---- graph -> core assignment (greedy balance on edge load) ----
    cnt_g = np.bincount(batch, minlength=N_GRAPHS).astype(np.int64)
    eload = np.bincount(batch[dst], minlength=N_GRAPHS).astype(np.int64)
    order = np.argsort(-eload, kind="stable")
    core_e = np.zeros(M, np.int64)
    core_n = np.zeros(M, np.int64)
    core_g = np.zeros(M, np.int64)
    graph_core = np.zeros(N_GRAPHS, np.int64)
    node_cap = int(np.ceil(N / M / 128.0 + 1)) * 128
    for g in order:
        cands = np.nonzero((core_n + cnt_g[g] <= node_cap)
                           & (core_g < 128))[0]
        if len(cands) == 0:
            cands = np.nonzero(core_g < 128)[0]
        c = cands[np.argmin(core_e[cands])]
        graph_core[g] = c
        core_e[c] += eload[g]
        core_n[c] += cnt_g[g]
        core_g[c] += 1
    T = int(np.ceil(core_n.max() / 128.0))
    T2 = (T + 1) // 2    # int16 row-index limit: 4*T2*128 <= 32767
    S = T * 128

    core_graphs = [np.nonzero(graph_core == c)[0] for c in range(M)]
    node_core = graph_core[batch]

    # ---- node -> (tile, pos) within core: balance per-tile in-edge load ----
    indeg = np.bincount(dst, minlength=N).astype(np.int64)  # includes self
    tile_of = np.zeros(N, np.int64)
    pos_of = np.zeros(N, np.int64)
    glocal = np.zeros(N, np.int64)
    import heapq
    for c in range(M):
        gl = {g: i for i, g in enumerate(core_graphs[c])}
        nodes = np.nonzero(node_core == c)[0]
        glocal[nodes] = [gl[g] for g in batch[nodes]]
        nd = nodes[np.argsort(-indeg[nodes], kind="stable")]
        heap = [(0, 0, t) for t in range(T)]
        heapq.heapify(heap)
        counts = np.zeros(T, np.int64)
        for n in nd:
            load, cnt, t = heapq.heappop(heap)
            tile_of[n] = t
            pos_of[n] = counts[t]
            counts[t] += 1
            if counts[t] < 128:
                heapq.heappush(heap, (load + indeg[n], counts[t], t))

    # ---- zfull row / region layout (2 collective chunks x 2 core halves) ---
    n_chunk0 = M * T2 * 128
    tpc1 = T - T2

    def zrow(nodes):
        c = node_core[nodes]
        t = tile_of[nodes]
        p = pos_of[nodes]
        in0 = t < T2
        r = np.where(in0,
                     c * (T2 * 128) + t * 128 + p,
                     n_chunk0 + c * (tpc1 * 128) + (t - T2) * 128 + p)
        return r

    reg_sizes = np.array([4 * T2 * 128, 4 * T2 * 128,
                          4 * tpc1 * 128, 4 * tpc1 * 128], np.int64)
    reg_base = np.concatenate([[0], np.cumsum(reg_sizes)])[:4]
    assert reg_sizes.max() <= 32767, reg_sizes

    def region(nodes):
        c = node_core[nodes]
        t = tile_of[nodes]
        return np.where(t < T2, 0, 2) + (c >= 4).astype(np.int64)

    src_row = zrow(src)
    src_reg = region(src)
    nrm_e = (dis[src] * dis[dst]).astype(np.float32)
    e_core = node_core[dst]
    e_tile = tile_of[dst]
    e_pos = pos_of[dst]

    # ---- group edges by (core, tile, region); common block counts ----
    NC = int(np.ceil(T / TPC))
    counts = np.zeros((M, T, 4), np.int64)
    np.add.at(counts, (e_core, e_tile, src_reg), 1)
    B_tr = np.ceil(counts.max(axis=0) / 128.0).astype(np.int64)  # [T, 4]
    B_tr = np.maximum(B_tr, 1)

    # global block order: chunk-major, then region, then tile, then block
    blk_cols = {}
    col = 0
    nblk_chunk = np.zeros(NC, np.int64)
    for ch in range(NC):
        t0, t1 = ch * TPC, min((ch + 1) * TPC, T)
        base = col
        for r in range(4):
            for t in range(t0, t1):
                blk_cols[(t, r)] = col
                col += B_tr[t, r]
        nblk_chunk[ch] = col - base
    TOTB = col
    NBLKC = int(nblk_chunk.max())
    TOTS = TOTB * 128

    # call plan: per (chunk, region) a list of (colbase, nblk<=8) subcalls
    calls = []
    for ch in range(NC):
        t0, t1 = ch * TPC, min((ch + 1) * TPC, T)
        cb = int(nblk_chunk[:ch].sum())
        for r in range(4):
            nb = int(B_tr[t0:t1, r].sum())
            first = blk_cols[(t0, r)]
            done = 0
            while done < nb:
                k = min(nb - done, MAXIDX // 128)
                calls.append(dict(chunk=ch, region=r,
                                  goff=first - cb + done - 0,
                                  gcol=first + done, nblk=k))
                done += k

    # per-tile matmul block lists (chunk-local g columns)
    tile_blocks = []  # [t] -> list of chunk-local block offsets
    for t in range(T):
        ch = t // TPC
        cb = int(nblk_chunk[:ch].sum())
        bl = []
        for r in range(4):
            c0 = blk_cols[(t, r)]
            bl.extend(range(c0 - cb, c0 - cb + int(B_tr[t, r])))
        tile_blocks.append(bl)

    # ---- per-core slot planes ----
    in_maps = []
    for c in range(M):
        sel = e_core == c
        et, er, epos_c = e_tile[sel], src_reg[sel], e_pos[sel]
        esrc_local = (src_row[sel] - reg_base[er]).astype(np.int64)
        enrm = nrm_e[sel]
        okey = et * 4 + er
        order_e = np.argsort(okey, kind="stable")
        et, er, epos_c = et[order_e], er[order_e], epos_c[order_e]
        esrc_local, enrm = esrc_local[order_e], enrm[order_e]
        cnt_tr = np.zeros((T, 4), np.int64)
        np.add.at(cnt_tr, (et, er), 1)
        cum = np.zeros((T, 4), np.int64)
        flat = cnt_tr.reshape(-1)
        cumflat = np.concatenate([[0], np.cumsum(flat)])[:-1]
        cum = cumflat.reshape(T, 4)
        # slot index within (t, r) group:
        within = np.arange(len(et)) - cum[et, er]
        gblk = np.array([blk_cols[(int(t), int(r))] for t, r in zip(et, er)])
        slot = gblk * 128 + within
        idxplane = np.zeros(TOTS, np.int64)
        nrmplane = np.zeros(TOTS, np.float32)
        ldstplane = np.full(TOTS, -1.0, np.float32)
        idxplane[slot] = esrc_local
        nrmplane[slot] = enrm
        ldstplane[slot] = epos_c

        # idx wrapped layout [128, TOTS//16] int16, replicated per 16-part grp
        wrapped = idxplane.reshape(TOTS // 16, 16).T.astype(np.int16)
        idx_sb = np.zeros((128, TOTS // 16), np.int16)
        for k in range(8):
            idx_sb[16 * k:16 * (k + 1), :] = wrapped

        def to_cols(a):  # slot plane -> [128, TOTB] (partition = slot % 128)
            return np.ascontiguousarray(a.reshape(TOTB, 128).T)

        nodes = np.nonzero(node_core == c)[0]
        xT = np.zeros((128, S), np.float32)
        scol = tile_of[nodes] * 128 + pos_of[nodes]
        xT[:, scol] = x[nodes].T
        gcolT = np.full((128, T), -1.0, np.float32)
        gcolT[pos_of[nodes], tile_of[nodes]] = glocal[nodes]
        discol = np.zeros((128, T), np.float32)
        discol[pos_of[nodes], tile_of[nodes]] = dis[nodes]

        gpc = len(core_graphs[c])
        invcnt = np.zeros((128, 1), np.float32)
        invcnt[:gpc, 0] = 1.0 / np.maximum(cnt_g[core_graphs[c]], 1.0)

        in_maps.append(dict(
            xT=_bf16(xT),
            idxp=idx_sb,
            nrmp=_bf16(to_cols(nrmplane)),
            ldstp=_bf16(to_cols(ldstplane)),
            gcolT=_bf16(gcolT),
            invcnt=invcnt.astype(np.float32),
        ))

    iota = np.broadcast_to(np.arange(128, dtype=np.float32), (128, 128)).copy()
    ident = np.eye(128, dtype=np.float32)
    shared = dict(iota=_bf16(iota), ident=ident)

    meta = dict(T=T, T2=T2, S=S, NC=NC, TOTB=TOTB, TOTS=TOTS, NBLKC=NBLKC,
                nblk_chunk=nblk_chunk.tolist(), calls=calls,
                tile_blocks=tile_blocks, reg_base=reg_base.tolist(),
                reg_sizes=reg_sizes.tolist(), n_chunk0=n_chunk0,
                core_graphs=core_graphs, shared=shared)
    return in_maps, meta


def make_weight_inputs(W1, b1, Wh, bh, Wout, bout):
    b14 = np.stack([np.asarray(b1, np.float32)] +
                   [np.asarray(bh[i], np.float32) for i in range(3)], axis=1)
    return dict(
        W1=_bf16(W1),
        Wh2=_bf16(np.concatenate([np.asarray(Wh[i]) for i in range(4)],
                                 axis=1)),
        b14=np.ascontiguousarray(b14, np.float32),
        b5rep=np.broadcast_to(np.asarray(bh[3], np.float32), (128, H)).copy(),
        woutrep=np.broadcast_to(np.asarray(Wout, np.float32)[:, 0],
                                (128, H)).copy(),
    ), float(np.asarray(bout, np.float32)[0])


# ---------------------------------------------------------------------------
# Bass program
# ---------------------------------------------------------------------------

def build_nc(meta, bout):
    from concourse.library_config import mlp

    T, T2, S, NC = meta["T"], meta["T2"], meta["S"], meta["NC"]
    TOTB, TOTS, NBLKC = meta["TOTB"], meta["TOTS"], meta["NBLKC"]
    nblk_chunk = meta["nblk_chunk"]
    calls = meta["calls"]
    tile_blocks = meta["tile_blocks"]
    reg_base = meta["reg_base"]
    reg_sizes = meta["reg_sizes"]
    n_chunk0 = meta["n_chunk0"]
    ZR = M * S

    nc = bass.Bass("TRN2", target_bir_lowering=False, num_swdge_queues=4)

    xT_d = nc.declare_dram_parameter("xT", [128, S], BF16, isOutput=False)
    idxp_d = nc.declare_dram_parameter("idxp", [128, TOTS // 16], I16, isOutput=False)
    nrmp_d = nc.declare_dram_parameter("nrmp", [128, TOTB], BF16, isOutput=False)
    ldstp_d = nc.declare_dram_parameter("ldstp", [128, TOTB], BF16, isOutput=False)
    gcolT_d = nc.declare_dram_parameter("gcolT", [128, T], BF16, isOutput=False)
    invcnt_d = nc.declare_dram_parameter("invcnt", [128, 1], F32, isOutput=False)
    iota_d = nc.declare_dram_parameter("iota", [128, 128], BF16, isOutput=False)
    ident_d = nc.declare_dram_parameter("ident", [128, 128], F32, isOutput=False)
    W1_d = nc.declare_dram_parameter("W1", [128, H], BF16, isOutput=False)
    Wh2_d = nc.declare_dram_parameter("Wh2", [H, 4 * H], BF16, isOutput=False)
    b14_d = nc.declare_dram_parameter("b14", [H, 4], F32, isOutput=False)
    b5rep_d = nc.declare_dram_parameter("b5rep", [128, H], F32, isOutput=False)
    woutrep_d = nc.declare_dram_parameter("woutrep", [128, H], F32, isOutput=False)
    out_d = nc.declare_dram_parameter("out", [128, 1], F32, isOutput=True)

    with tile.TileContext(nc) as tc:
        with contextlib.ExitStack() as ctx:
            dram = ctx.enter_context(tc.tile_pool(name="dram", bufs=1, space="DRAM"))
            zpool = ctx.enter_context(tc.tile_pool(name="zpool", bufs=1, space="DRAM"))
            const = ctx.enter_context(tc.tile_pool(name="const", bufs=1))
            xp = ctx.enter_context(tc.tile_pool(name="xp", bufs=3))
            gp = ctx.enter_context(tc.tile_pool(name="gp", bufs=4))
            mp = ctx.enter_context(tc.tile_pool(name="mp", bufs=2))
            hp = ctx.enter_context(tc.tile_pool(name="hp", bufs=3))
            zp = ctx.enter_context(tc.tile_pool(name="zp", bufs=3))
            ep = ctx.enter_context(tc.tile_pool(name="ep", bufs=2))
            ps_agg = ctx.enter_context(tc.tile_pool(name="ps_agg", bufs=3, space="PSUM"))
            ps_z = ctx.enter_context(tc.tile_pool(name="ps_z", bufs=2, space="PSUM"))
            ps_pool = ctx.enter_context(tc.tile_pool(name="ps_pool", bufs=1, space="PSUM"))

            nc.gpsimd.load_library(mlp)

            zfullsA = [zpool.tile([n_chunk0, 128], BF16, addr_space="Shared",
                                  name=f"zfullA{k}", tag=f"zfullA{k}")
                       for k in range(5)]
            zfullsB = [zpool.tile([ZR - n_chunk0, 128], BF16,
                                  addr_space="Shared",
                                  name=f"zfullB{k}", tag=f"zfullB{k}")
                       for k in range(5)]
            bounces = [dram.tile([S, 128], BF16, name=f"bounce{k}",
                                 tag=f"bounce{k}") for k in range(5)]

            idx_sb = const.tile([128, TOTS // 16], I16)
            nc.sync.dma_start(idx_sb[:], idxp_d[:])
            nrm_sb = const.tile([128, TOTB], BF16)
            nc.sync.dma_start(nrm_sb[:], nrmp_d[:])
            ldst_sb = const.tile([128, TOTB], BF16)
            nc.sync.dma_start(ldst_sb[:], ldstp_d[:])
            gcol_sb = const.tile([128, T], BF16)
            nc.sync.dma_start(gcol_sb[:], gcolT_d[:])
            invcnt_sb = const.tile([128, 1], F32)
            nc.sync.dma_start(invcnt_sb[:], invcnt_d[:])
            iota_sb = const.tile([128, 128], BF16)
            nc.sync.dma_start(iota_sb[:], iota_d[:])
            ident_sb = const.tile([128, 128], F32)
            nc.sync.dma_start(ident_sb[:], ident_d[:])
            W1_sb = const.tile([128, H], BF16)
            nc.sync.dma_start(W1_sb[:], W1_d[:])
            Wh2_sb = const.tile([H, 4 * H], BF16)
            nc.sync.dma_start(Wh2_sb[:], Wh2_d[:])
            b14_sb = const.tile([H, 4], F32)
            nc.sync.dma_start(b14_sb[:], b14_d[:])
            b5rep_sb = const.tile([128, H], F32)
            nc.sync.dma_start(b5rep_sb[:], b5rep_d[:])
            woutrep_sb = const.tile([128, H], F32)
            nc.sync.dma_start(woutrep_sb[:], woutrep_d[:])

            nblk_vals = sorted({cl["nblk"] for cl in calls})
            nidx_regs = {nb: nc.gpsimd.to_reg(nb * 128) for nb in nblk_vals}

            def all_gather_A(layer):
                bo = bounces[layer]
                nc.gpsimd.collective_compute(
                    "AllGather", mybir.AluOpType.bypass,
                    replica_groups=[list(range(M))],
                    ins=[bo[0:T2 * 128, :]], outs=[zfullsA[layer][:]])

            def all_gather_B(layer):
                bo = bounces[layer]
                if T > T2:
                    nc.gpsimd.collective_compute(
                        "AllGather", mybir.AluOpType.bypass,
                        replica_groups=[list(range(M))],
                        ins=[bo[T2 * 128:S, :]], outs=[zfullsB[layer][:]])

            # ---- layer 0: z1 = x @ W1 per tile ----
            for t in range(T):
                xt = xp.tile([128, 128], BF16, tag="xt")
                nc.sync.dma_start(xt[:], xT_d[:, t * 128:(t + 1) * 128])
                pz = ps_z.tile([128, H], F32, space="PSUM", tag="pz")
                nc.tensor.matmul(out=pz[:], lhsT=xt[:], rhs=W1_sb[:],
                                 start=True, stop=True)
                zt = zp.tile([128, H], BF16, tag="zt")
                nc.scalar.copy(zt[:], pz[:])
                nc.sync.dma_start(bounces[0][t * 128:(t + 1) * 128, 0:H], zt[:])
                if t == T2 - 1:
                    all_gather_A(0)
            all_gather_B(0)

            # ---- layers 1..5 ----
            chunk_call = [[cl for cl in calls if cl["chunk"] == ch]
                          for ch in range(NC)]
            qrr = [0]  # round-robin queue cursor (queue choice is free)

            def emit_call(layer, ch, g, cl):
                cb = int(np.sum(nblk_chunk[:ch]))
                r = cl["region"]
                go = cl["gcol"] - cb
                if r < 2:
                    zf, rb = zfullsA[layer - 1], reg_base[r]
                else:
                    zf, rb = zfullsB[layer - 1], reg_base[r] - n_chunk0
                rs = reg_sizes[r]
                nc.gpsimd.dma_gather(
                    g[:, go:go + cl["nblk"], :],
                    zf[rb:rb + rs, :],
                    idx_sb[:, cl["gcol"] * 8:(cl["gcol"] + cl["nblk"]) * 8],
                    cl["nblk"] * 128, nidx_regs[cl["nblk"]], 128,
                    single_packet=True, queue_num=qrr[0])
                qrr[0] = (qrr[0] + 1) % 4

            def issue_calls(layer, specs):
                for ch, g, regions in specs:
                    for cl in chunk_call[ch]:
                        if cl["region"] in regions:
                            emit_call(layer, ch, g, cl)

            AHEAD = 2  # chunks of regions-0/1 gathers issued ahead
            for layer in range(1, 6):
                if layer == 5:
                    ppool = ps_pool.tile([H, 128], F32, space="PSUM", tag="pp")
                gtiles = {}
                for step in range(NC + AHEAD):
                    specs = []
                    if step < NC:
                        g = gp.tile([128, NBLKC, 128], BF16, tag="g")
                        gtiles[step] = g
                        specs.append((step, g, (0, 1)))
                    if step >= AHEAD:
                        specs.append((step - AHEAD, gtiles[step - AHEAD], (2, 3)))
                    issue_calls(layer, specs)
                    if step < AHEAD:
                        continue
                    ch = step - AHEAD
                    g = gtiles.pop(ch)
                    cb = int(np.sum(nblk_chunk[:ch]))
                    nblk = int(nblk_chunk[ch])
                    # norm scale on live halves
                    g3 = g[:, 0:nblk, 0:H]
                    nrm3 = nrm_sb[:, cb:cb + nblk, None].to_broadcast(
                        [128, nblk, H])
                    nc.vector.tensor_tensor(out=g3, in0=g3, in1=nrm3,
                                            op=mybir.AluOpType.mult)
                    # one-hot build for the whole chunk
                    m01 = mp.tile([128, NBLKC, 128], BF16, tag="m01")
                    m3 = m01[:, 0:nblk, :]
                    iota3 = iota_sb[:, None, :].to_broadcast([128, nblk, 128])
                    ldst3 = ldst_sb[:, cb:cb + nblk, None].to_broadcast(
                        [128, nblk, 128])
                    nc.vector.tensor_tensor(out=m3, in0=iota3, in1=ldst3,
                                            op=mybir.AluOpType.is_equal)
                    t0, t1 = ch * TPC, min((ch + 1) * TPC, T)
                    for t in range(t0, t1):
                        bl = tile_blocks[t]
                        if layer < 5:
                            pT = ps_agg.tile([H, 128], F32, space="PSUM", tag="pT")
                            for i, b in enumerate(bl):
                                nc.tensor.matmul(
                                    out=pT[:], lhsT=g[:, b, 0:H],
                                    rhs=m01[:, b, :],
                                    start=(i == 0), stop=(i == len(bl) - 1))
                            hT = hp.tile([H, 128], BF16, tag="hT")
                            nc.scalar.activation(
                                hT[:], pT[:], mybir.ActivationFunctionType.Relu,
                                bias=b14_sb[:, layer - 1:layer])
                            pz = ps_z.tile([128, H], F32, space="PSUM", tag="pz")
                            nc.tensor.matmul(
                                out=pz[:], lhsT=hT[:],
                                rhs=Wh2_sb[:, (layer - 1) * H:layer * H],
                                start=True, stop=True)
                            zt = zp.tile([128, H], BF16, tag="zt2")
                            nc.scalar.copy(zt[:], pz[:])
                            nc.sync.dma_start(
                                bounces[layer][t * 128:(t + 1) * 128, 0:H], zt[:])
                        else:
                            p5 = ps_agg.tile([128, H], F32, space="PSUM", tag="pT")
                            for i, b in enumerate(bl):
                                nc.tensor.matmul(
                                    out=p5[:], lhsT=m01[:, b, :],
                                    rhs=g[:, b, 0:H],
                                    start=(i == 0), stop=(i == len(bl) - 1))
                            h5 = hp.tile([128, H], BF16, tag="h5")
                            nc.vector.tensor_tensor(
                                out=h5[:], in0=p5[:], in1=b5rep_sb[:],
                                op=mybir.AluOpType.add)
                            nc.scalar.activation(
                                h5[:], h5[:], mybir.ActivationFunctionType.Relu)
                            pt = mp.tile([128, 128], BF16, tag="pt")
                            nc.vector.tensor_tensor(
                                out=pt[:], in0=iota_sb[:],
                                in1=gcol_sb[:, t:t + 1].to_broadcast([128, 128]),
                                op=mybir.AluOpType.is_equal)
                            nc.tensor.matmul(out=ppool[:], lhsT=h5[:], rhs=pt[:],
                                             start=(t == 0), stop=(t == T - 1))
                    if layer < 5 and ch == (T2 + TPC - 1) // TPC - 1:
                        all_gather_A(layer)
                if layer < 5:
                    all_gather_B(layer)

            # ---- pool tail: mean, layernorm, head (fp32) ----
            pool_sb = ep.tile([H, 128], F32, tag="poolsb")
            nc.scalar.copy(pool_sb[:], ppool[:])
            ptr = ps_z.tile([128, H], F32, space="PSUM", tag="ptr")
            nc.tensor.transpose(out=ptr[:], in_=pool_sb[:],
                                identity=ident_sb[0:H, 0:H])
            pooled = ep.tile([128, H], F32, tag="pooled")
            nc.vector.tensor_scalar(out=pooled[:], in0=ptr[:],
                                    scalar1=invcnt_sb[:, 0:1], scalar2=None,
                                    op0=mybir.AluOpType.mult)
            mu = ep.tile([128, 1], F32, tag="mu")
            nc.vector.tensor_reduce(out=mu[:], in_=pooled[:],
                                    axis=mybir.AxisListType.X,
                                    op=mybir.AluOpType.add)
            nc.vector.tensor_scalar(out=mu[:], in0=mu[:], scalar1=1.0 / H,
                                    scalar2=None, op0=mybir.AluOpType.mult)
            xc = ep.tile([128, H], F32, tag="xc")
            nc.vector.tensor_scalar(out=xc[:], in0=pooled[:], scalar1=mu[:, 0:1],
                                    scalar2=None, op0=mybir.AluOpType.subtract)
            sq = ep.tile([128, H], F32, tag="sq")
            nc.scalar.activation(sq[:], xc[:], mybir.ActivationFunctionType.Square)
            var = ep.tile([128, 1], F32, tag="var")
            nc.vector.tensor_reduce(out=var[:], in_=sq[:],
                                    axis=mybir.AxisListType.X,
                                    op=mybir.AluOpType.add)
            nc.vector.tensor_scalar(out=var[:], in0=var[:], scalar1=1.0 / H,
                                    scalar2=None, op0=mybir.AluOpType.mult)
            eps_col = ep.tile([128, 1], F32, tag="eps")
            nc.vector.memset(eps_col[:], 1e-5)
            std = ep.tile([128, 1], F32, tag="std")
            nc.scalar.activation(std[:], var[:], mybir.ActivationFunctionType.Sqrt,
                                 bias=eps_col[:, 0:1])
            rstd = ep.tile([128, 1], F32, tag="rstd")
            nc.vector.reciprocal(rstd[:], std[:])
            ln = ep.tile([128, H], F32, tag="ln")
            nc.vector.tensor_scalar(out=ln[:], in0=xc[:], scalar1=rstd[:, 0:1],
                                    scalar2=None, op0=mybir.AluOpType.mult)
            y = ep.tile([128, H], F32, tag="y")
            nc.vector.tensor_tensor(out=y[:], in0=ln[:], in1=woutrep_sb[:],
                                    op=mybir.AluOpType.mult)
            yr = ep.tile([128, 1], F32, tag="yr")
            nc.vector.tensor_reduce(out=yr[:], in_=y[:],
                                    axis=mybir.AxisListType.X,
                                    op=mybir.AluOpType.add)
            nc.vector.tensor_scalar(out=yr[:], in0=yr[:], scalar1=bout,
                                    scalar2=None, op0=mybir.AluOpType.add)
            nc.sync.dma_start(out_d[:], yr[:])
    return nc


# ---------------------------------------------------------------------------
# Entry point
# ---------------------------------------------------------------------------

def kernel(x, edge_index, batch, W1, b1, Wh, bh, Wout, bout):
    from concourse.bass_utils import run_bass_kernel_spmd
    from concourse.library_overlay import lower_extended_insts

    x = np.asarray(x, np.float32)
    edge_index = np.asarray(edge_index)
    batch = np.asarray(batch)

    in_maps, meta = preprocess(x, edge_index, batch)
    wmaps, bout_v = make_weight_inputs(W1, b1, Wh, bh, Wout, bout)
    nc = build_nc(meta, bout_v)
    lower_extended_insts(nc)
    for im in in_maps:
        im.update(wmaps)
        im.update(meta["shared"])

    import time
    last_err = None
    for attempt in range(3):
        try:
            res = run_bass_kernel_spmd(nc, in_maps, core_ids=list(range(M)))
            break
        except Exception as e:  # transient terminal hiccups / device recovery
            last_err = e
            time.sleep(30 * (attempt + 1))
    else:
        raise last_err

    out = np.zeros((N_GRAPHS, 1), np.float32)
    for c in range(M):
        gl = meta["core_graphs"][c]
        out[gl, 0] = res.results[c]["out"][:len(gl), 0]
    return np.ascontiguousarray(out, np.float32)


# revision 34
# speedup vs baseline: 1.4152x; 1.3411x over previous
"""GCN-5 message-passing kernel for Trainium2, 8-core SPMD Bass/Tile.

Strategy (graph-parallel per the sharding hint):
  - Graphs are assigned to the 8 cores by a greedy balance on edge load;
    each core owns whole graphs, its nodes, and the edges terminating in
    them (plus their self-loops).  Pool/LN/head are local per core.
  - Per layer each core computes z = h @ W for its node slice (bf16,
    rows padded to 256B), the slices are AllGathered in two row-chunks
    (overlapping the producing tile loop), and each core gathers z[src]
    rows for its edges with gpsimd dma_gather: batched descriptor
    generation (<=1024 rows per call, single_packet) spread across the 4
    SWDGE queues so all four Q7 pairs generate descriptors in parallel.
  - Scatter-add per 128-node tile is a one-hot matmul on the
    TensorEngine with PSUM accumulation (edges pre-sorted by dst tile /
    src region and padded to 128-slot blocks on the host; padded slots
    carry nrm=0 so they contribute nothing).  The symmetric norm
    deg^-1/2(src)*deg^-1/2(dst) is a per-slot bf16 plane folded into one
    vector multiply on the gathered rows.
  - All matmul operands are bf16 (PSUM accumulation fp32); gather and
    collective payloads are bf16.  Final pool/LN/head tail is fp32.
"""
import sys
import types
import contextlib

import numpy as np
import ml_dtypes

sys.path.insert(0, "/opt/trn_rl_repo")

import concourse.bass as bass
import concourse.tile as tile
from concourse import mybir
from concourse.vector_clock import ScopedClock

F32 = mybir.dt.float32
BF16 = mybir.dt.bfloat16
I16 = mybir.dt.int16
M = 8   # NeuronCores
H = 64
N_GRAPHS = 1000
TPC = 8       # dst tiles per gather chunk
MAXIDX = 1024  # max rows per dma_gather call (16KB/engine packet limit)

# ---------------------------------------------------------------------------
# Environment fixes for this container
# ---------------------------------------------------------------------------

def _install_env_fixes():
    import concourse.tile as tile_mod

    def _patched_drain_and_barrier(self, tick_clock, wait_clock):
        # this walrus build allows a single sync-wait per TPB_CTRL Drain;
        # split the Tile tail-drain's waits across multiple drains.
        nc = self.nc
        drain_inst = nc.sync.drain()
        wait_clock.add_sem_waits(drain_inst.ins,
                                 ScopedClock({None: tick_clock.global_clock}))
        si = drain_inst.ins.sync_info
        waits = list(si.on_wait or [])
        if len(waits) > 1:
            si.on_wait[:] = waits[:1]
            for w in waits[1:]:
                d2 = nc.sync.drain()
                if d2.ins.sync_info is None:
                    d2.ins.sync_info = mybir.SyncInfo(on_wait=[w], on_update=[])
                else:
                    d2.ins.sync_info.on_wait.append(w)
        nc.all_engine_barrier()
        assert self.sems is not None
        popped = nc._tile_sem_poison_stack.pop()
        assert popped is self._sem_poison
        nc.clear_and_free_semaphores(list(self.sems.allocated().values()))
        nc.all_engine_barrier()

    tile_mod.TileContext._drain_and_barrier = _patched_drain_and_barrier

    _orig_lower_ordered = tile_mod.TileContext._lower_ordered_insts

    def _split_multiwait_lower(self, ordered):
        nc = self.nc
        for bbname, insts in ordered.items():
            newlist = []
            changed = False
            for inst in insts:
                si = getattr(inst, "sync_info", None)
                eng = getattr(inst, "engine", None)
                if (si is not None and si.on_wait and len(si.on_wait) > 1
                        and eng is not None and eng != mybir.EngineType.Unassigned
                        and inst.is_executable()):
                    waits = list(si.on_wait)
                    si.on_wait[:] = waits[-1:]
                    for w in waits[:-1]:
                        nop = mybir.InstNoOp(
                            name=nc.get_next_instruction_name(), engine=eng)
                        nop.sync_info = mybir.SyncInfo(on_wait=[w], on_update=[])
                        try:
                            nc.register_instruction(nop, overwrite=True)
                        except Exception:
                            pass
                        newlist.append(nop)
                    changed = True
                newlist.append(inst)
            if changed:
                insts[:] = newlist
        return _orig_lower_ordered(self, ordered)

    if getattr(tile_mod.TileContext._lower_ordered_insts, "__name__", "") != \
            "_split_multiwait_lower":
        tile_mod.TileContext._lower_ordered_insts = _split_multiwait_lower

    # NTFF profile hook (lets trace=True work under axon); best-effort.
    if "antenv.axon_hooks" not in sys.modules:
        try:
            from trn_agent_boot.trn_boot import _ntff_profile_via_ctypes
            hook = _ntff_profile_via_ctypes("/opt/axon/libaxon_pjrt.so")
            mod = types.ModuleType("antenv.axon_hooks")
            mod.get_axon_ntff_profile_hook = lambda: hook
            mod.set_axon_ntff_profile_hook = lambda h: None
            sys.modules["antenv.axon_hooks"] = mod
            import antenv
            antenv.axon_hooks = mod
        except Exception:
            pass


_install_env_fixes()


def _bf16(a):
    return np.ascontiguousarray(np.asarray(a, np.float32)).astype(ml_dtypes.bfloat16)


# ---------------------------------------------------------------------------
# Host preprocessing
# ---------------------------------------------------------------------------

def preprocess(x, edge_index, batch):
    N = x.shape[0]
    src0 = edge_index[0].astype(np.int64)
    dst0 = edge_index[1].astype(np.int64)
    loops = np.arange(N, dtype=np.int64)
    src = np.concatenate([src0, loops])
    dst = np.concatenate([dst0, loops])
    deg = np.bincount(dst, minlength=N).astype(np.float32)
    dis = 1.0 / np.sqrt(deg)          # deg >= 1 (self loops)
    batch = np.asarray(batch).astype(np.int64)

    # ---- graph -> core assignment (greedy balance on edge load) ----
    cnt_g = np.bincount(batch, minlength=N_GRAPHS).astype(np.int64)
    eload = np.bincount(batch[dst], minlength=N_GRAPHS).astype(np.int64)
    order = np.argsort(-eload, kind="stable")
    core_e = np.zeros(M, np.int64)
    core_n = np.zeros(M, np.int64)
    core_g = np.zeros(M, np.int64)
    graph_core = np.zeros(N_GRAPHS, np.int64)
    node_cap = int(np.ceil(N / M / 128.0 + 1)) * 128
    for g in order:
        cands = np.nonzero((core_n + cnt_g[g] <= node_cap)
                           & (core_g < 128))[0]
        if len(cands) == 0:
            cands = np.nonzero(core_g < 128)[0]
        c = cands[np.argmin(core_e[cands])]
        graph_core[g] = c
        core_e[c] += eload[g]
        core_n[c] += cnt_g[g]
        core_g[c] += 1
    T = int(np.ceil(core_n.max() / 128.0))
    T2 = (T + 1) // 2    # int16 row-index limit: 4*T2*128 <= 32767
    S = T * 128

    core_graphs = [np.nonzero(graph_core == c)[0] for c in range(M)]
    node_core = graph_core[batch]

    # ---- node -> (tile, pos) within core: balance per-tile in-edge load ----
    indeg = np.bincount(dst, minlength=N).astype(np.int64)  # includes self
    tile_of = np.zeros(N, np.int64)
    pos_of = np.zeros(N, np.int64)
    glocal = np.zeros(N, np.int64)
    import heapq
    for c in range(M):
        gl = {g: i for i, g in enumerate(core_graphs[c])}
        nodes = np.nonzero(node_core == c)[0]
        glocal[nodes] = [gl[g] for g in batch[nodes]]
        nd = nodes[np.argsort(-indeg[nodes], kind="stable")]
        heap = [(0, 0, t) for t in range(T)]
        heapq.heapify(heap)
        counts = np.zeros(T, np.int64)
        for n in nd:
            load, cnt, t = heapq.heappop(heap)
            tile_of[n] = t
            pos_of[n] = counts[t]
            counts[t] += 1
            if counts[t] < 128:
                heapq.heappush(heap, (load + indeg[n], counts[t], t))

    # pass 2: re-balance tiles on the per-region in-edge 4-vector (regions
    # derived from the pass-1 assignment; second-order shifts are ignored)
    reg1 = np.where(tile_of < T2, 0, 2) + (node_core >= 4).astype(np.int64)
    nodereg = np.zeros((N, 4), np.int64)
    ns = src != dst
    np.add.at(nodereg, (dst[ns], reg1[src[ns]]), 1)
    for c in range(M):
        nodes = np.nonzero(node_core == c)[0]
        loads = np.zeros((T, 4), np.float64)
        counts = np.zeros(T, np.int64)
        for half in (0, 1):
            lo, hi = (0, T2) if half == 0 else (T2, T)
            hn = nodes[(tile_of[nodes] >= lo) & (tile_of[nodes] < hi)]
            hn = hn[np.argsort(-indeg[hn], kind="stable")]
            for n in hn:
                v = nodereg[n]
                proj = loads[lo:hi] + v
                score = proj.max(axis=1) + 1e-3 * proj.sum(axis=1) \
                    + 1e18 * (counts[lo:hi] >= 128)
                t = lo + int(np.argmin(score))
                tile_of[n] = t
                pos_of[n] = counts[t]
                counts[t] += 1
                loads[t] += v

    # ---- zfull row / region layout (2 collective chunks x 2 core halves) ---
    n_chunk0 = M * T2 * 128
    tpc1 = T - T2

    def zrow(nodes):
        c = node_core[nodes]
        t = tile_of[nodes]
        p = pos_of[nodes]
        in0 = t < T2
        r = np.where(in0,
                     c * (T2 * 128) + t * 128 + p,
                     n_chunk0 + c * (tpc1 * 128) + (t - T2) * 128 + p)
        return r

    reg_sizes = np.array([4 * T2 * 128, 4 * T2 * 128,
                          4 * tpc1 * 128, 4 * tpc1 * 128], np.int64)
    reg_base = np.concatenate([[0], np.cumsum(reg_sizes)])[:4]
    assert reg_sizes.max() <= 32767, reg_sizes

    def region(nodes):
        c = node_core[nodes]
        t = tile_of[nodes]
        return np.where(t < T2, 0, 2) + (c >= 4).astype(np.int64)

    # self-loop edges are handled as per-tile "self blocks" (contiguous
    # static DMA + identity one-hot), not as gather slots
    nonself = src != dst
    src_row = zrow(src[nonself])
    src_reg = region(src[nonself])
    nrm_e = (dis[src[nonself]] * dis[dst[nonself]]).astype(np.float32)
    e_core = node_core[dst[nonself]]
    e_tile = tile_of[dst[nonself]]
    e_pos = pos_of[dst[nonself]]

    # ---- group edges by (core, tile, region); common block counts ----
    NC = int(np.ceil(T / TPC))
    counts = np.zeros((M, T, 4), np.int64)
    np.add.at(counts, (e_core, e_tile, src_reg), 1)
    B_tr = np.ceil(counts.max(axis=0) / 128.0).astype(np.int64)  # [T, 4]
    B_tr = np.maximum(B_tr, 1)

    # global block order: chunk-major, then region, then tile, then block;
    # each chunk ends with one self block per tile
    blk_cols = {}
    self_col = {}
    col = 0
    nblk_chunk = np.zeros(NC, np.int64)
    for ch in range(NC):
        t0, t1 = ch * TPC, min((ch + 1) * TPC, T)
        base = col
        for r in range(4):
            for t in range(t0, t1):
                blk_cols[(t, r)] = col
                col += B_tr[t, r]
        for t in range(t0, t1):
            self_col[t] = col
            col += 1
        nblk_chunk[ch] = col - base
    TOTB = col
    NBLKC = int(nblk_chunk.max())
    TOTS = TOTB * 128

    # call plan: per (chunk, region) a list of (colbase, nblk<=8) subcalls
    calls = []
    for ch in range(NC):
        t0, t1 = ch * TPC, min((ch + 1) * TPC, T)
        cb = int(nblk_chunk[:ch].sum())
        for r in range(4):
            nb = int(B_tr[t0:t1, r].sum())
            first = blk_cols[(t0, r)]
            done = 0
            while done < nb:
                k = min(nb - done, MAXIDX // 128)
                calls.append(dict(chunk=ch, region=r,
                                  goff=first - cb + done - 0,
                                  gcol=first + done, nblk=k))
                done += k

    # per-tile matmul block lists (chunk-local g columns), self block last
    tile_blocks = []  # [t] -> list of chunk-local block offsets
    self_cols = []    # [t] -> chunk-local self block offset
    for t in range(T):
        ch = t // TPC
        cb = int(nblk_chunk[:ch].sum())
        bl = []
        for r in range(4):
            c0 = blk_cols[(t, r)]
            bl.extend(range(c0 - cb, c0 - cb + int(B_tr[t, r])))
        bl.append(self_col[t] - cb)
        tile_blocks.append(bl)
        self_cols.append(self_col[t] - cb)

    # ---- per-core slot planes ----
    in_maps = []
    for c in range(M):
        sel = e_core == c
        et, er, epos_c = e_tile[sel], src_reg[sel], e_pos[sel]
        esrc_local = (src_row[sel] - reg_base[er]).astype(np.int64)
        enrm = nrm_e[sel]
        okey = et * 4 + er
        order_e = np.argsort(okey, kind="stable")
        et, er, epos_c = et[order_e], er[order_e], epos_c[order_e]
        esrc_local, enrm = esrc_local[order_e], enrm[order_e]
        cnt_tr = np.zeros((T, 4), np.int64)
        np.add.at(cnt_tr, (et, er), 1)
        cum = np.zeros((T, 4), np.int64)
        flat = cnt_tr.reshape(-1)
        cumflat = np.concatenate([[0], np.cumsum(flat)])[:-1]
        cum = cumflat.reshape(T, 4)
        # slot index within (t, r) group:
        within = np.arange(len(et)) - cum[et, er]
        gblk = np.array([blk_cols[(int(t), int(r))] for t, r in zip(et, er)])
        slot = gblk * 128 + within
        idxplane = np.zeros(TOTS, np.int64)
        nrmplane = np.zeros(TOTS, np.float32)
        ldstplane = np.full(TOTS, -1.0, np.float32)
        idxplane[slot] = esrc_local
        nrmplane[slot] = enrm
        ldstplane[slot] = epos_c
        # self blocks: identity one-hot, nrm = dis^2 (pads stay -1 / 0)
        cn = np.nonzero(node_core == c)[0]
        sslot = np.array([self_col[t] for t in tile_of[cn]]) * 128 + pos_of[cn]
        ldstplane[sslot] = pos_of[cn]
        nrmplane[sslot] = dis[cn] * dis[cn]

        # idx wrapped layout [128, TOTS//16] int16, replicated per 16-part grp
        wrapped = idxplane.reshape(TOTS // 16, 16).T.astype(np.int16)
        idx_sb = np.zeros((128, TOTS // 16), np.int16)
        for k in range(8):
            idx_sb[16 * k:16 * (k + 1), :] = wrapped

        def to_cols(a):  # slot plane -> [128, TOTB] (partition = slot % 128)
            return np.ascontiguousarray(a.reshape(TOTB, 128).T)

        nodes = np.nonzero(node_core == c)[0]
        xT = np.zeros((128, S), np.float32)
        scol = tile_of[nodes] * 128 + pos_of[nodes]
        xT[:, scol] = x[nodes].T
        gcolT = np.full((128, T), -1.0, np.float32)
        gcolT[pos_of[nodes], tile_of[nodes]] = glocal[nodes]
        discol = np.zeros((128, T), np.float32)
        discol[pos_of[nodes], tile_of[nodes]] = dis[nodes]

        gpc = len(core_graphs[c])
        invcnt = np.zeros((128, 1), np.float32)
        invcnt[:gpc, 0] = 1.0 / np.maximum(cnt_g[core_graphs[c]], 1.0)

        in_maps.append(dict(
            xT=_bf16(xT),
            idxp=idx_sb,
            nrmp=_bf16(to_cols(nrmplane)),
            ldstp=_bf16(to_cols(ldstplane)),
            gcolT=_bf16(gcolT),
            invcnt=invcnt.astype(np.float32),
        ))

    iota = np.broadcast_to(np.arange(128, dtype=np.float32), (128, 128)).copy()
    ident = np.eye(128, dtype=np.float32)
    shared = dict(iota=_bf16(iota), ident=ident)

    meta = dict(T=T, T2=T2, S=S, NC=NC, TOTB=TOTB, TOTS=TOTS, NBLKC=NBLKC,
                nblk_chunk=nblk_chunk.tolist(), calls=calls,
                tile_blocks=tile_blocks, self_cols=self_cols,
                reg_base=reg_base.tolist(),
                reg_sizes=reg_sizes.tolist(), n_chunk0=n_chunk0,
                core_graphs=core_graphs, shared=shared)
    return in_maps, meta


def make_weight_inputs(W1, b1, Wh, bh, Wout, bout):
    b14 = np.stack([np.asarray(b1, np.float32)] +
                   [np.asarray(bh[i], np.float32) for i in range(3)], axis=1)
    return dict(
        W1=_bf16(W1),
        Wh2=_bf16(np.concatenate([np.asarray(Wh[i]) for i in range(4)],
                                 axis=1)),
        b14=np.ascontiguousarray(b14, np.float32),
        b5rep=np.broadcast_to(np.asarray(bh[3], np.float32), (128, H)).copy(),
        woutrep=np.broadcast_to(np.asarray(Wout, np.float32)[:, 0],
                                (128, H)).copy(),
    ), float(np.asarray(bout, np.float32)[0])


# ---------------------------------------------------------------------------
# Bass program
# ---------------------------------------------------------------------------

def build_nc(meta, bout):
    from concourse.library_config import mlp

    T, T2, S, NC = meta["T"], meta["T2"], meta["S"], meta["NC"]
    TOTB, TOTS, NBLKC = meta["TOTB"], meta["TOTS"], meta["NBLKC"]
    nblk_chunk = meta["nblk_chunk"]
    calls = meta["calls"]
    tile_blocks = meta["tile_blocks"]
    self_cols = meta["self_cols"]
    reg_base = meta["reg_base"]
    reg_sizes = meta["reg_sizes"]
    n_chunk0 = meta["n_chunk0"]
    ZR = M * S

    nc = bass.Bass("TRN2", target_bir_lowering=False, num_swdge_queues=4)

    xT_d = nc.declare_dram_parameter("xT", [128, S], BF16, isOutput=False)
    idxp_d = nc.declare_dram_parameter("idxp", [128, TOTS // 16], I16, isOutput=False)
    nrmp_d = nc.declare_dram_parameter("nrmp", [128, TOTB], BF16, isOutput=False)
    ldstp_d = nc.declare_dram_parameter("ldstp", [128, TOTB], BF16, isOutput=False)
    gcolT_d = nc.declare_dram_parameter("gcolT", [128, T], BF16, isOutput=False)
    invcnt_d = nc.declare_dram_parameter("invcnt", [128, 1], F32, isOutput=False)
    iota_d = nc.declare_dram_parameter("iota", [128, 128], BF16, isOutput=False)
    ident_d = nc.declare_dram_parameter("ident", [128, 128], F32, isOutput=False)
    W1_d = nc.declare_dram_parameter("W1", [128, H], BF16, isOutput=False)
    Wh2_d = nc.declare_dram_parameter("Wh2", [H, 4 * H], BF16, isOutput=False)
    b14_d = nc.declare_dram_parameter("b14", [H, 4], F32, isOutput=False)
    b5rep_d = nc.declare_dram_parameter("b5rep", [128, H], F32, isOutput=False)
    woutrep_d = nc.declare_dram_parameter("woutrep", [128, H], F32, isOutput=False)
    out_d = nc.declare_dram_parameter("out", [128, 1], F32, isOutput=True)

    with tile.TileContext(nc) as tc:
        with contextlib.ExitStack() as ctx:
            dram = ctx.enter_context(tc.tile_pool(name="dram", bufs=1, space="DRAM"))
            zpool = ctx.enter_context(tc.tile_pool(name="zpool", bufs=1, space="DRAM"))
            const = ctx.enter_context(tc.tile_pool(name="const", bufs=1))
            xp = ctx.enter_context(tc.tile_pool(name="xp", bufs=3))
            gp = ctx.enter_context(tc.tile_pool(name="gp", bufs=4))
            mp = ctx.enter_context(tc.tile_pool(name="mp", bufs=2))
            hp = ctx.enter_context(tc.tile_pool(name="hp", bufs=3))
            zp = ctx.enter_context(tc.tile_pool(name="zp", bufs=3))
            ep = ctx.enter_context(tc.tile_pool(name="ep", bufs=2))
            ps_agg = ctx.enter_context(tc.tile_pool(name="ps_agg", bufs=3, space="PSUM"))
            ps_z = ctx.enter_context(tc.tile_pool(name="ps_z", bufs=2, space="PSUM"))
            ps_pool = ctx.enter_context(tc.tile_pool(name="ps_pool", bufs=1, space="PSUM"))

            nc.gpsimd.load_library(mlp)

            zfullsA = [zpool.tile([n_chunk0, 128], BF16, addr_space="Shared",
                                  name=f"zfullA{k}", tag=f"zfullA{k}")
                       for k in range(5)]
            zfullsB = [zpool.tile([ZR - n_chunk0, 128], BF16,
                                  addr_space="Shared",
                                  name=f"zfullB{k}", tag=f"zfullB{k}")
                       for k in range(5)]
            bounces = [dram.tile([S, 128], BF16, name=f"bounce{k}",
                                 tag=f"bounce{k}") for k in range(5)]

            idx_sb = const.tile([128, TOTS // 16], I16)
            nc.sync.dma_start(idx_sb[:], idxp_d[:])
            nrm_sb = const.tile([128, TOTB], BF16)
            nc.sync.dma_start(nrm_sb[:], nrmp_d[:])
            ldst_sb = const.tile([128, TOTB], BF16)
            nc.sync.dma_start(ldst_sb[:], ldstp_d[:])
            gcol_sb = const.tile([128, T], BF16)
            nc.sync.dma_start(gcol_sb[:], gcolT_d[:])
            invcnt_sb = const.tile([128, 1], F32)
            nc.sync.dma_start(invcnt_sb[:], invcnt_d[:])
            iota_sb = const.tile([128, 128], BF16)
            nc.sync.dma_start(iota_sb[:], iota_d[:])
            ident_sb = const.tile([128, 128], F32)
            nc.sync.dma_start(ident_sb[:], ident_d[:])
            W1_sb = const.tile([128, H], BF16)
            nc.sync.dma_start(W1_sb[:], W1_d[:])
            Wh2_sb = const.tile([H, 4 * H], BF16)
            nc.sync.dma_start(Wh2_sb[:], Wh2_d[:])
            b14_sb = const.tile([H, 4], F32)
            nc.sync.dma_start(b14_sb[:], b14_d[:])
            b5rep_sb = const.tile([128, H], F32)
            nc.sync.dma_start(b5rep_sb[:], b5rep_d[:])
            woutrep_sb = const.tile([128, H], F32)
            nc.sync.dma_start(woutrep_sb[:], woutrep_d[:])

            nblk_vals = sorted({cl["nblk"] for cl in calls})
            nidx_regs = {nb: nc.gpsimd.to_reg(nb * 128) for nb in nblk_vals}

            def all_gather_A(layer):
                bo = bounces[layer]
                nc.gpsimd.collective_compute(
                    "AllGather", mybir.AluOpType.bypass,
                    replica_groups=[list(range(M))],
                    ins=[bo[0:T2 * 128, :]], outs=[zfullsA[layer][:]])

            def all_gather_B(layer):
                bo = bounces[layer]
                if T > T2:
                    nc.gpsimd.collective_compute(
                        "AllGather", mybir.AluOpType.bypass,
                        replica_groups=[list(range(M))],
                        ins=[bo[T2 * 128:S, :]], outs=[zfullsB[layer][:]])

            # ---- layer 0: z1 = x @ W1 per tile ----
            for t in range(T):
                xt = xp.tile([128, 128], BF16, tag="xt")
                nc.sync.dma_start(xt[:], xT_d[:, t * 128:(t + 1) * 128])
                pz = ps_z.tile([128, H], F32, space="PSUM", tag="pz")
                nc.tensor.matmul(out=pz[:], lhsT=xt[:], rhs=W1_sb[:],
                                 start=True, stop=True)
                zt = zp.tile([128, H], BF16, tag="zt")
                nc.scalar.copy(zt[:], pz[:])
                nc.sync.dma_start(bounces[0][t * 128:(t + 1) * 128, 0:H], zt[:])
                if t == T2 - 1:
                    all_gather_A(0)
            all_gather_B(0)

            # ---- layers 1..5 ----
            chunk_call = [[cl for cl in calls if cl["chunk"] == ch]
                          for ch in range(NC)]
            qrr = [0]  # round-robin queue cursor (queue choice is free)

            def emit_call(layer, ch, g, cl):
                cb = int(np.sum(nblk_chunk[:ch]))
                r = cl["region"]
                go = cl["gcol"] - cb
                if r < 2:
                    zf, rb = zfullsA[layer - 1], reg_base[r]
                else:
                    zf, rb = zfullsB[layer - 1], reg_base[r] - n_chunk0
                rs = reg_sizes[r]
                nc.gpsimd.dma_gather(
                    g[:, go:go + cl["nblk"], :],
                    zf[rb:rb + rs, :],
                    idx_sb[:, cl["gcol"] * 8:(cl["gcol"] + cl["nblk"]) * 8],
                    cl["nblk"] * 128, nidx_regs[cl["nblk"]], 128,
                    single_packet=True, queue_num=qrr[0])
                qrr[0] = (qrr[0] + 1) % 4

            def issue_calls(layer, specs):
                for ch, g, regions in specs:
                    for cl in chunk_call[ch]:
                        if cl["region"] in regions:
                            emit_call(layer, ch, g, cl)

            AHEAD = 2  # chunks of regions-0/1 gathers issued ahead
            for layer in range(1, 6):
                if layer == 5:
                    ppool = ps_pool.tile([H, 128], F32, space="PSUM", tag="pp")
                gtiles = {}
                for step in range(NC + AHEAD):
                    specs = []
                    if step < NC:
                        g = gp.tile([128, NBLKC, 128], BF16, tag="g")
                        gtiles[step] = g
                        specs.append((step, g, (0, 1)))
                    if step >= AHEAD:
                        specs.append((step - AHEAD, gtiles[step - AHEAD], (2, 3)))
                    issue_calls(layer, specs)
                    if step < AHEAD:
                        continue
                    ch = step - AHEAD
                    g = gtiles.pop(ch)
                    cb = int(np.sum(nblk_chunk[:ch]))
                    nblk = int(nblk_chunk[ch])
                    # self blocks: contiguous rows from own bounce (static DMA,
                    # no gpsimd descriptor cost)
                    for t in range(ch * TPC, min((ch + 1) * TPC, T)):
                        nc.sync.dma_start(
                            g[:, self_cols[t], :],
                            bounces[layer - 1][t * 128:(t + 1) * 128, :])
                    # one-hot build first: no gather dependency, overlaps the
                    # in-flight gathers on the DVE
                    m01 = mp.tile([128, NBLKC, 128], BF16, tag="m01")
                    m3 = m01[:, 0:nblk, :]
                    iota3 = iota_sb[:, None, :].to_broadcast([128, nblk, 128])
                    ldst3 = ldst_sb[:, cb:cb + nblk, None].to_broadcast(
                        [128, nblk, 128])
                    nc.vector.tensor_tensor(out=m3, in0=iota3, in1=ldst3,
                                            op=mybir.AluOpType.is_equal)
                    # norm scale on live halves
                    g3 = g[:, 0:nblk, 0:H]
                    nrm3 = nrm_sb[:, cb:cb + nblk, None].to_broadcast(
                        [128, nblk, H])
                    nc.vector.tensor_tensor(out=g3, in0=g3, in1=nrm3,
                                            op=mybir.AluOpType.mult)
                    t0, t1 = ch * TPC, min((ch + 1) * TPC, T)
                    for t in range(t0, t1):
                        bl = tile_blocks[t]
                        if layer < 5:
                            pT = ps_agg.tile([H, 128], F32, space="PSUM", tag="pT")
                            for i, b in enumerate(bl):
                                nc.tensor.matmul(
                                    out=pT[:], lhsT=g[:, b, 0:H],
                                    rhs=m01[:, b, :],
                                    start=(i == 0), stop=(i == len(bl) - 1))
                            hT = hp.tile([H, 128], BF16, tag="hT")
                            nc.scalar.activation(
                                hT[:], pT[:], mybir.ActivationFunctionType.Relu,
                                bias=b14_sb[:, layer - 1:layer])
                            pz = ps_z.tile([128, H], F32, space="PSUM", tag="pz")
                            nc.tensor.matmul(
                                out=pz[:], lhsT=hT[:],
                                rhs=Wh2_sb[:, (layer - 1) * H:layer * H],
                                start=True, stop=True)
                            zt = zp.tile([128, H], BF16, tag="zt2")
                            nc.scalar.copy(zt[:], pz[:])
                            nc.sync.dma_start(
                                bounces[layer][t * 128:(t + 1) * 128, 0:H], zt[:])
                        else:
                            p5 = ps_agg.tile([128, H], F32, space="PSUM", tag="pT")
                            for i, b in enumerate(bl):
                                nc.tensor.matmul(
                                    out=p5[:], lhsT=m01[:, b, :],
                                    rhs=g[:, b, 0:H],
                                    start=(i == 0), stop=(i == len(bl) - 1))
                            h5 = hp.tile([128, H], BF16, tag="h5")
                            nc.vector.tensor_tensor(
                                out=h5[:], in0=p5[:], in1=b5rep_sb[:],
                                op=mybir.AluOpType.add)
                            nc.scalar.activation(
                                h5[:], h5[:], mybir.ActivationFunctionType.Relu)
                            pt = mp.tile([128, 128], BF16, tag="pt")
                            nc.vector.tensor_tensor(
                                out=pt[:], in0=iota_sb[:],
                                in1=gcol_sb[:, t:t + 1].to_broadcast([128, 128]),
                                op=mybir.AluOpType.is_equal)
                            nc.tensor.matmul(out=ppool[:], lhsT=h5[:], rhs=pt[:],
                                             start=(t == 0), stop=(t == T - 1))
                    if layer < 5 and ch == (T2 + TPC - 1) // TPC - 1:
                        all_gather_A(layer)
                if layer < 5:
                    all_gather_B(layer)

            # ---- pool tail: mean, layernorm, head (fp32) ----
            pool_sb = ep.tile([H, 128], F32, tag="poolsb")
            nc.scalar.copy(pool_sb[:], ppool[:])
            ptr = ps_z.tile([128, H], F32, space="PSUM", tag="ptr")
            nc.tensor.transpose(out=ptr[:], in_=pool_sb[:],
                                identity=ident_sb[0:H, 0:H])
            pooled = ep.tile([128, H], F32, tag="pooled")
            nc.vector.tensor_scalar(out=pooled[:], in0=ptr[:],
                                    scalar1=invcnt_sb[:, 0:1], scalar2=None,
                                    op0=mybir.AluOpType.mult)
            mu = ep.tile([128, 1], F32, tag="mu")
            nc.vector.tensor_reduce(out=mu[:], in_=pooled[:],
                                    axis=mybir.AxisListType.X,
                                    op=mybir.AluOpType.add)
            nc.vector.tensor_scalar(out=mu[:], in0=mu[:], scalar1=1.0 / H,
                                    scalar2=None, op0=mybir.AluOpType.mult)
            xc = ep.tile([128, H], F32, tag="xc")
            nc.vector.tensor_scalar(out=xc[:], in0=pooled[:], scalar1=mu[:, 0:1],
                                    scalar2=None, op0=mybir.AluOpType.subtract)
            sq = ep.tile([128, H], F32, tag="sq")
            nc.scalar.activation(sq[:], xc[:], mybir.ActivationFunctionType.Square)
            var = ep.tile([128, 1], F32, tag="var")
            nc.vector.tensor_reduce(out=var[:], in_=sq[:],
                                    axis=mybir.AxisListType.X,
                                    op=mybir.AluOpType.add)
            nc.vector.tensor_scalar(out=var[:], in0=var[:], scalar1=1.0 / H,
                                    scalar2=None, op0=mybir.AluOpType.mult)
            eps_col = ep.tile([128, 1], F32, tag="eps")
            nc.vector.memset(eps_col[:], 1e-5)
            std = ep.tile([128, 1], F32, tag="std")
            nc.scalar.activation(std[:], var[:], mybir.ActivationFunctionType.Sqrt,
                                 bias=eps_col[:, 0:1])
            rstd = ep.tile([128, 1], F32, tag="rstd")
            nc.vector.reciprocal(rstd[:], std[:])
            ln = ep.tile([128, H], F32, tag="ln")
            nc.vector.tensor_scalar(out=ln[:], in0=xc[:], scalar1=rstd[:, 0:1],
                                    scalar2=None, op0=mybir.AluOpType.mult)
            y = ep.tile([128, H], F32, tag="y")
            nc.vector.tensor_tensor(out=y[:], in0=ln[:], in1=woutrep_sb[:],
                                    op=mybir.AluOpType.mult)
            yr = ep.tile([128, 1], F32, tag="yr")
            nc.vector.tensor_reduce(out=yr[:], in_=y[:],
                                    axis=mybir.AxisListType.X,
                                    op=mybir.AluOpType.add)
            nc.vector.tensor_scalar(out=yr[:], in0=yr[:], scalar1=bout,
                                    scalar2=None, op0=mybir.AluOpType.add)
            nc.sync.dma_start(out_d[:], yr[:])
    return nc


# ---------------------------------------------------------------------------
# Entry point
# ---------------------------------------------------------------------------

def kernel(x, edge_index, batch, W1, b1, Wh, bh, Wout, bout):
    from concourse.bass_utils import run_bass_kernel_spmd
    from concourse.library_overlay import lower_extended_insts

    x = np.asarray(x, np.float32)
    edge_index = np.asarray(edge_index)
    batch = np.asarray(batch)

    in_maps, meta = preprocess(x, edge_index, batch)
    wmaps, bout_v = make_weight_inputs(W1, b1, Wh, bh, Wout, bout)
    nc = build_nc(meta, bout_v)
    lower_extended_insts(nc)
    for im in in_maps:
        im.update(wmaps)
        im.update(meta["shared"])

    import time
    last_err = None
    for attempt in range(3):
        try:
            res = run_bass_kernel_spmd(nc, in_maps, core_ids=list(range(M)))
            break
        except Exception as e:  # transient terminal hiccups / device recovery
            last_err = e
            time.sleep(30 * (attempt + 1))
    else:
        raise last_err

    out = np.zeros((N_GRAPHS, 1), np.float32)
    for c in range(M):
        gl = meta["core_graphs"][c]
        out[gl, 0] = res.results[c]["out"][:len(gl), 0]
    return np.ascontiguousarray(out, np.float32)
